# revision 3
# baseline (speedup 1.0000x reference)
"""Chunked cross-attention (RETRO-style) Trainium2 kernel.

Full-input contract: kernel(**inputs) takes the unsharded tensors and returns
the full [B, S, D] output. Internally shards (batch, chunk-half) across 8
NeuronCores: core r handles batch r//2, chunks (r%2)*16..(r%2)*16+16.

Per-core device program. All four dense projections (q/k/v/out) run as
fp8e4 DoubleRow matmuls (K=256 per instruction, 2x PE throughput); weights
are cast f32->fp8 directly in the load DMA (gpsimd SWDGE). Attention
(scores, softmax, attn@v) stays bf16. Absolute-scale analysis: inputs are
~N(0,1) and weights Xavier-small, so direct e4m3 casts keep the final
output inside the 2e-2 relative-error gate (measured ~1.3e-2).

Pipeline: chunk-pair p's dense projections are emitted before pair p-2's
attention so the PE never starves; the LayerNorm+q-projection phase is
slotted after pair 1's projections so the x DMA + LN vector work hides
under pair-0/1 e-transposes and k/v matmuls.

The v bias is NOT added to v2; because attention rows are normalized
(sum_j a_ij = 1), attn @ (v + bv) = attn @ v + bv, so bv is added as a
per-partition scalar during the PSUM->SBUF diagonal-block copy of the
attention output (zero extra instructions).
"""

import numpy as np

import concourse.bacc as bacc
import concourse.bass as bass
import concourse.mybir as mybir
import concourse.tile as tile
from concourse.bass_utils import run_bass_kernel_spmd

F32 = mybir.dt.float32
BF16 = mybir.dt.bfloat16
FP8 = mybir.dt.float8e4
DR = mybir.MatmulPerfMode.DoubleRow

B, S, D = 4, 2048, 1024
C, N, L = 32, 2, 128
H, DK = 16, 64
CHUNK = 64
EPS = 1e-5
SCALE = 1.0 / np.sqrt(DK)

HDK = H * DK          # 1024
KC = D // 128         # 8 contraction chunks
MC = HDK // 128       # 8 output chunks
CPC = C // 2          # 16 chunks per core
TOK = N * L           # 256 neighbor tokens per chunk
R = CPC * CHUNK       # 1024 query rows per core
HP = H // 2           # 8 head pairs
PAIRS = CPC // 2      # 8 chunk pairs

Exp = mybir.ActivationFunctionType.Exp
Sqrt = mybir.ActivationFunctionType.Sqrt
SUB = mybir.AluOpType.subtract
MULT = mybir.AluOpType.mult
ADD = mybir.AluOpType.add


def build_bass():
    nc = bacc.Bacc(None, target_bir_lowering=False, debug=False)

    x = nc.dram_tensor("x", [R, D], F32, kind="ExternalInput").ap()
    ev = nc.dram_tensor("ev", [CPC * TOK, D], F32, kind="ExternalInput").ap()
    Wq = nc.dram_tensor("Wq", [D, HDK], F32, kind="ExternalInput").ap()
    Wk = nc.dram_tensor("Wk", [D, HDK], F32, kind="ExternalInput").ap()
    Wv = nc.dram_tensor("Wv", [D, HDK], F32, kind="ExternalInput").ap()
    Wo = nc.dram_tensor("Wo", [HDK, D], F32, kind="ExternalInput").ap()
    bq = nc.dram_tensor("bq", [HDK], F32, kind="ExternalInput").ap()
    bk = nc.dram_tensor("bk", [HDK], F32, kind="ExternalInput").ap()
    bv = nc.dram_tensor("bv", [HDK], F32, kind="ExternalInput").ap()
    bo = nc.dram_tensor("bo", [D], F32, kind="ExternalInput").ap()
    gamma = nc.dram_tensor("gamma", [D], F32, kind="ExternalInput").ap()
    beta = nc.dram_tensor("beta", [D], F32, kind="ExternalInput").ap()
    y = nc.dram_tensor("y", [R, D], F32, kind="ExternalOutput").ap()

    def bcast(ap):
        # view a [D] dram vector as [128, D] (partition-broadcast read)
        return bass.AP(tensor=ap.tensor, offset=ap.offset, ap=[[0, 128]] + list(ap.ap))

    from contextlib import ExitStack
    with tile.TileContext(nc) as tc, ExitStack() as ctx:
        wts = ctx.enter_context(tc.tile_pool(name="wts", bufs=4))
        qtp = ctx.enter_context(tc.tile_pool(name="qtp", bufs=1))
        cons = ctx.enter_context(tc.tile_pool(name="cons", bufs=1))
        xrow = ctx.enter_context(tc.tile_pool(name="xrow", bufs=3))
        stat = ctx.enter_context(tc.tile_pool(name="stat", bufs=4))
        xnp = ctx.enter_context(tc.tile_pool(name="xnp", bufs=2))
        xbp = ctx.enter_context(tc.tile_pool(name="xbp", bufs=2))
        esb = ctx.enter_context(tc.tile_pool(name="esb", bufs=2))
        etp = ctx.enter_context(tc.tile_pool(name="etp", bufs=2))
        ktp = ctx.enter_context(tc.tile_pool(name="ktp", bufs=3))
        vsb = ctx.enter_context(tc.tile_pool(name="vsb", bufs=3))
        atp = ctx.enter_context(tc.tile_pool(name="atp", bufs=4))
        attp = ctx.enter_context(tc.tile_pool(name="attp", bufs=4))
        aotp = ctx.enter_context(tc.tile_pool(name="aotp", bufs=2))
        ysb = ctx.enter_context(tc.tile_pool(name="ysb", bufs=2))
        rrp = ctx.enter_context(tc.tile_pool(name="rrp", bufs=4))
        ps_pp = ctx.enter_context(tc.tile_pool(name="ps_pp", bufs=2, space="PSUM"))
        ps_sc = ctx.enter_context(tc.tile_pool(name="ps_sc", bufs=2, space="PSUM"))
        ps_ov = ctx.enter_context(tc.tile_pool(name="ps_ov", bufs=2, space="PSUM"))
        ps_tr = ctx.enter_context(tc.tile_pool(name="ps_tr", bufs=2, space="PSUM"))

        # ---- e prefetch first: pair 0/1 bf16 tiles feed the first PE work ----
        ev_v = ev.rearrange("(pr cc nj p) d -> pr p cc nj d", pr=PAIRS, cc=2, p=128)
        e2_tiles = {}
        for pr in range(2):
            e2t = esb.tile([128, 2, N, D], BF16, tag="e")
            e2_tiles[pr] = e2t
            nc.gpsimd.dma_start(out=e2t, in_=ev_v[pr])

        # ---- weights: direct f32 -> fp8e4 cast in the load DMA ----
        Wk_sb = wts.tile([128, KC, HDK], FP8, tag="w")
        nc.gpsimd.dma_start(out=Wk_sb, in_=Wk.rearrange("(kc p) n -> p kc n", p=128))
        Wv_sb = wts.tile([128, KC, HDK], FP8, tag="w")
        nc.gpsimd.dma_start(out=Wv_sb, in_=Wv.rearrange("(kc p) n -> p kc n", p=128))

        # ---- constants ----
        from concourse.masks import make_identity
        identB = cons.tile([128, 128], BF16)
        make_identity(nc, identB)
        bqc = cons.tile([128, MC], F32)
        nc.sync.dma_start(out=bqc, in_=bq.rearrange("(f p) -> p f", p=128))
        bkc = cons.tile([128, MC], F32)
        nc.sync.dma_start(out=bkc, in_=bk.rearrange("(f p) -> p f", p=128))
        bvc = cons.tile([128, MC], F32)
        nc.sync.dma_start(out=bvc, in_=bv.rearrange("(f p) -> p f", p=128))
        boB = cons.tile([128, D], F32)
        nc.gpsimd.dma_start(out=boB, in_=bcast(bo))
        gammaB = cons.tile([128, D], F32)
        nc.gpsimd.dma_start(out=gammaB, in_=bcast(gamma))
        betaB = cons.tile([128, D], F32)
        nc.gpsimd.dma_start(out=betaB, in_=bcast(beta))
        epsT = cons.tile([128, 1], F32)
        nc.vector.memset(epsT, EPS)

        Wq_sb = wts.tile([128, KC, HDK], FP8, tag="w")
        nc.gpsimd.dma_start(out=Wq_sb, in_=Wq.rearrange("(kc p) n -> p kc n", p=128))

        kv_tiles = {}

        def emit_proj(pr):
            if pr in e2_tiles:
                e2 = e2_tiles.pop(pr)
            else:
                e2 = esb.tile([128, 2, N, D], BF16, tag="e")
                nc.gpsimd.dma_start(out=e2, in_=ev_v[pr])
            # PE transpose (bf16) -> PSUM; cast to fp8 in the PSUM->SBUF copy
            eT = etp.tile([128, KC, 2 * TOK], FP8, tag="eT")
            for kc in range(KC):
                for cc in range(2):
                    pt = ps_tr.tile([128, 2, 128], BF16, tag="pt")
                    for nj in range(N):
                        nc.tensor.transpose(pt[:, nj, :],
                                            e2[:, cc, nj, kc * 128:(kc + 1) * 128],
                                            identB)
                    nc.any.tensor_copy(
                        out=eT[:, kc, cc * TOK:(cc + 1) * TOK], in_=pt)

            kT = ktp.tile([128, MC, 2, TOK], BF16, tag="kT")
            for m in range(MC):
                pk = ps_pp.tile([128, 512], F32, tag="pp")
                for kk in range(KC // 2):
                    nc.tensor.matmul(pk, Wk_sb[:, 2 * kk:2 * kk + 2,
                                               m * 128:(m + 1) * 128],
                                     eT[:, 2 * kk:2 * kk + 2, :],
                                     start=(kk == 0), stop=(kk == KC // 2 - 1),
                                     perf_mode=DR)
                nc.vector.tensor_scalar(out=kT[:, m, :, :], in0=pk.rearrange(
                    "p (cc t) -> p cc t", cc=2),
                    scalar1=bkc[:, m:m + 1], scalar2=None, op0=ADD)

            # v2 WITHOUT bias (bv folded into the attention-output copy)
            v2 = vsb.tile([128, 2, N, H, DK], BF16, tag="v")
            for cc in range(2):
                for nj in range(N):
                    for n in range(2):
                        pv = ps_pp.tile([128, 512], F32, tag="pp")
                        for kk in range(KC // 2):
                            nc.tensor.matmul(
                                pv,
                                eT[:, 2 * kk:2 * kk + 2,
                                   cc * TOK + nj * 128:cc * TOK + (nj + 1) * 128],
                                Wv_sb[:, 2 * kk:2 * kk + 2, n * 512:(n + 1) * 512],
                                start=(kk == 0), stop=(kk == KC // 2 - 1),
                                perf_mode=DR)
                        nc.any.tensor_copy(
                            out=v2[:, cc, nj, n * 8:(n + 1) * 8, :],
                            in_=pv.rearrange("p (h d) -> p h d", h=8))
            kv_tiles[pr] = (kT, v2)

        # ---- phase A: LN + transpose + q projection (emitted after pair-1
        # projections; x DMA + LN vector work overlaps pair-0/1 PE work) ----
        def emit_phase_a():
            xnT = wts.tile([128, KC, R], FP8, tag="w")
            for rt in range(R // 128):
                xa = xrow.tile([128, D], F32, tag="xrow")
                nc.sync.dma_start(out=xa, in_=x[rt * 128:(rt + 1) * 128, :])
                stats = stat.tile([128, 2, 6], F32, tag="st")
                for sg in range(2):
                    nc.vector.bn_stats(out=stats[:, sg, :],
                                       in_=xa[:, sg * 512:(sg + 1) * 512])
                mv = stat.tile([128, 2], F32, tag="mv")
                nc.vector.bn_aggr(out=mv, in_=stats)
                rstd = stat.tile([128, 1], F32, tag="rs")
                nc.scalar.activation(out=rstd, in_=mv[:, 1:2], func=Sqrt,
                                     bias=epsT, scale=1.0)
                nc.vector.reciprocal(out=rstd, in_=rstd)
                xn = xnp.tile([128, D], F32, tag="xn")
                nc.vector.tensor_scalar(out=xn, in0=xa, scalar1=mv[:, 0:1],
                                        scalar2=rstd, op0=SUB, op1=MULT)
                nc.vector.tensor_mul(out=xn, in0=xn, in1=gammaB)
                xnb = xbp.tile([128, D], BF16, tag="xnb")
                nc.vector.tensor_add(out=xnb, in0=xn, in1=betaB)
                for kc2 in range(KC // 2):
                    pt = ps_tr.tile([128, 2, 128], BF16, tag="pt")
                    for j in range(2):
                        kc = kc2 * 2 + j
                        nc.tensor.transpose(pt[:, j, :],
                                            xnb[:, kc * 128:(kc + 1) * 128], identB)
                    nc.any.tensor_copy(
                        out=xnT[:, kc2 * 2:kc2 * 2 + 2, rt * 128:(rt + 1) * 128],
                        in_=pt)

            qT = qtp.tile([128, MC, R], BF16)
            for m in range(MC):
                for n in range(2):
                    pq = ps_pp.tile([128, 512], F32, tag="pp")
                    for kk in range(KC // 2):
                        nc.tensor.matmul(pq, Wq_sb[:, 2 * kk:2 * kk + 2,
                                                   m * 128:(m + 1) * 128],
                                         xnT[:, 2 * kk:2 * kk + 2,
                                             n * 512:(n + 1) * 512],
                                         start=(kk == 0), stop=(kk == KC // 2 - 1),
                                         perf_mode=DR)
                    nc.vector.tensor_scalar(out=qT[:, m, n * 512:(n + 1) * 512],
                                            in0=pq, scalar1=bqc[:, m:m + 1],
                                            scalar2=None, op0=ADD)
            return qT

        qT_holder = {}

        def emit_attn(pr):
            qT = qT_holder["qT"]
            kT, v2 = kv_tiles.pop(pr)
            aoT = aotp.tile([128, MC, 128], FP8, tag="aoT")
            for hp in range(HP):
                ov2 = ps_ov.tile([128, 2, 128], F32, tag="ov")
                for cc in range(2):
                    cl = pr * 2 + cc
                    sc = ps_sc.tile([128, TOK], F32, tag="sc")
                    nc.tensor.matmul(sc[0:64, :], qT[0:64, hp, cl * 64:(cl + 1) * 64],
                                     kT[0:64, hp, cc, :], start=True, stop=True)
                    nc.tensor.matmul(sc[64:128, :],
                                     qT[64:128, hp, cl * 64:(cl + 1) * 64],
                                     kT[64:128, hp, cc, :], start=True, stop=True)
                    at = atp.tile([128, TOK], BF16, tag="at")
                    rs = rrp.tile([128, 1], F32, tag="rs")
                    nc.scalar.activation(out=at, in_=sc, func=Exp, scale=SCALE,
                                         accum_out=rs)
                    rr = rrp.tile([128, 1], F32, tag="rr")
                    nc.vector.reciprocal(out=rr, in_=rs)
                    # normalize on gpsimd (SBUF->SBUF; gpsimd has no PSUM port)
                    nc.gpsimd.tensor_scalar(out=at, in0=at, scalar1=rr,
                                            scalar2=None, op0=MULT)
                    att = attp.tile([128, N, 128], BF16, tag="att")
                    pt = ps_tr.tile([128, 2, 128], BF16, tag="pt")
                    for nj in range(N):
                        nc.tensor.transpose(pt[:, nj, :],
                                            at[:, nj * 128:(nj + 1) * 128], identB)
                    nc.any.tensor_copy(out=att, in_=pt)
                    # both heads in one [128,128] matmul; off-diagonal blocks
                    # are cross-head garbage, only diagonal blocks copied out
                    for nj in range(N):
                        nc.tensor.matmul(
                            ov2[:, cc, :],
                            v2[:, cc, nj, hp * 2:hp * 2 + 2, :].rearrange(
                                "p h d -> p (h d)"),
                            att[:, nj, :],
                            start=(nj == 0), stop=(nj == N - 1))
                # diagonal-block copy for both chunks at once, + bv bias
                # (valid because att rows are normalized), fp8 output
                for h01 in range(2):
                    sl = slice(h01 * 64, (h01 + 1) * 64)
                    nc.vector.tensor_scalar(
                        out=aoT[sl, hp, :].rearrange("p (cc i) -> p cc i", cc=2),
                        in0=ov2[sl, :, sl],
                        scalar1=bvc[sl, hp:hp + 1], scalar2=None, op0=ADD)

            xres = xrow.tile([128, D], F32, tag="xrow")
            nc.sync.dma_start(out=xres, in_=x[pr * 128:(pr + 1) * 128, :])
            y_sb = ysb.tile([128, D], F32, tag="y")
            for n in range(2):
                py = ps_pp.tile([128, 512], F32, tag="pp")
                for kk in range(MC // 2):
                    nc.tensor.matmul(py, aoT[:, 2 * kk:2 * kk + 2, :],
                                     Wo_sb[:, 2 * kk:2 * kk + 2,
                                           n * 512:(n + 1) * 512],
                                     start=(kk == 0), stop=(kk == MC // 2 - 1),
                                     perf_mode=DR)
                nc.vector.tensor_add(out=y_sb[:, n * 512:(n + 1) * 512], in0=py,
                                     in1=boB[:, n * 512:(n + 1) * 512])
            nc.gpsimd.tensor_add(out=y_sb, in0=y_sb, in1=xres)
            nc.sync.dma_start(out=y[pr * 128:(pr + 1) * 128, :], in_=y_sb)

        # ---- schedule ----
        emit_proj(0)
        emit_proj(1)
        qT_holder["qT"] = emit_phase_a()
        # Wo reuses a weight slot (free after q-proj's xnT... keep separate tag)
        Wo_sb = wts.tile([128, MC, D], FP8, tag="wo")
        nc.gpsimd.dma_start(out=Wo_sb, in_=Wo.rearrange("(mc p) n -> p mc n", p=128))
        for pr in range(2, PAIRS):
            emit_proj(pr)
            emit_attn(pr - 2)
        emit_attn(PAIRS - 2)
        emit_attn(PAIRS - 1)

    nc.compile()
    return nc


_NC = None


def _get_nc():
    global _NC
    if _NC is None:
        _NC = build_bass()
    return _NC


def _shard_inputs(h, e, Wq, bq, Wk, bk, Wv, bv, Wo, bo, gamma, beta):
    shared = {"Wq": Wq, "Wk": Wk, "Wv": Wv, "Wo": Wo, "bq": bq, "bk": bk,
              "bv": bv, "bo": bo, "gamma": gamma, "beta": beta}
    in_maps = []
    for r in range(8):
        b, half = divmod(r, 2)
        c0 = half * CPC
        t0 = CHUNK - 1 + c0 * CHUNK
        rows = h[b, t0:min(t0 + R, S)]
        if rows.shape[0] < R:
            rows = np.concatenate(
                [rows, np.zeros((R - rows.shape[0], D), np.float32)], axis=0)
        evs = np.ascontiguousarray(e[b, c0:c0 + CPC].reshape(CPC * TOK, D))
        in_maps.append({"x": np.ascontiguousarray(rows), "ev": evs, **shared})
    return in_maps


# results of the most recent run (exec_time_ns etc.) for test harnesses
LAST_RESULTS = None
TRACE = False


def kernel(h, e, Wq, bq, Wk, bk, Wv, bv, Wo, bo, gamma, beta):
    global LAST_RESULTS
    args = [np.asarray(a, dtype=np.float32) for a in
            (h, e, Wq, bq, Wk, bk, Wv, bv, Wo, bo, gamma, beta)]
    h, e = args[0], args[1]
    nc = _get_nc()
    in_maps = _shard_inputs(*args)
    res = run_bass_kernel_spmd(nc, in_maps, core_ids=list(range(8)), trace=TRACE)
    LAST_RESULTS = res
    out = np.empty((B, S, D), np.float32)
    out[:, :CHUNK - 1] = h[:, :CHUNK - 1]
    for r in range(8):
        b, half = divmod(r, 2)
        c0 = half * CPC
        t0 = CHUNK - 1 + c0 * CHUNK
        n = min(R, S - t0)
        out[b, t0:t0 + n] = res.results[r]["y"][:n]
    return out


# revision 13
# speedup vs baseline: 2.0020x; 2.0020x over previous
"""Chunked cross-attention (RETRO-style) Trainium2 kernel.

Full-input contract: kernel(**inputs) takes the unsharded tensors and returns
the full [B, S, D] output. Internally shards (batch, chunk-half) across 8
NeuronCores: core r handles batch r//2, chunks (r%2)*16..(r%2)*16+16.

Per-core device program. All four dense projections (q/k/v/out) run as
fp8e4 DoubleRow matmuls (K=256 per instruction, 2x PE throughput); weights
are cast f32->fp8 directly in the load DMA (gpsimd SWDGE). Attention
(scores, softmax, attn@v) stays bf16. Absolute-scale analysis: inputs are
~N(0,1) and weights Xavier-small, so direct e4m3 casts keep the final
output inside the 2e-2 relative-error gate (measured ~1.3e-2).

Pipeline: chunk-pair p's dense projections are emitted before pair p-2's
attention so the PE never starves; the LayerNorm+q-projection phase is
slotted after pair 1's projections so the x DMA + LN vector work hides
under pair-0/1 e-transposes and k/v matmuls.

The v bias is NOT added to v2; because attention rows are normalized
(sum_j a_ij = 1), attn @ (v + bv) = attn @ v + bv, so bv is added as a
per-partition scalar during the PSUM->SBUF diagonal-block copy of the
attention output (zero extra instructions).
"""

import numpy as np

import concourse.bacc as bacc
import concourse.bass as bass
import concourse.mybir as mybir
import concourse.tile as tile
from concourse.bass_utils import run_bass_kernel_spmd

F32 = mybir.dt.float32
BF16 = mybir.dt.bfloat16
FP8 = mybir.dt.float8e4
DR = mybir.MatmulPerfMode.DoubleRow

B, S, D = 4, 2048, 1024
C, N, L = 32, 2, 128
H, DK = 16, 64
CHUNK = 64
EPS = 1e-5
SCALE = 1.0 / np.sqrt(DK)

HDK = H * DK          # 1024
KC = D // 128         # 8 contraction chunks
MC = HDK // 128       # 8 output chunks
CPC = C // 2          # 16 chunks per core
TOK = N * L           # 256 neighbor tokens per chunk
R = CPC * CHUNK       # 1024 query rows per core
HP = H // 2           # 8 head pairs
PAIRS = CPC // 2      # 8 chunk pairs

Exp = mybir.ActivationFunctionType.Exp
Sqrt = mybir.ActivationFunctionType.Sqrt
Ident = mybir.ActivationFunctionType.Identity
SUB = mybir.AluOpType.subtract
MULT = mybir.AluOpType.mult
ADD = mybir.AluOpType.add


def build_bass():
    nc = bacc.Bacc(None, target_bir_lowering=False, debug=False)

    x = nc.dram_tensor("x", [R, D], F32, kind="ExternalInput").ap()
    ev = nc.dram_tensor("ev", [CPC * TOK, D], F32, kind="ExternalInput").ap()
    Wq = nc.dram_tensor("Wq", [D, HDK], F32, kind="ExternalInput").ap()
    Wk = nc.dram_tensor("Wk", [D, HDK], F32, kind="ExternalInput").ap()
    Wv = nc.dram_tensor("Wv", [D, HDK], F32, kind="ExternalInput").ap()
    Wo = nc.dram_tensor("Wo", [HDK, D], F32, kind="ExternalInput").ap()
    bq = nc.dram_tensor("bq", [HDK], F32, kind="ExternalInput").ap()
    bk = nc.dram_tensor("bk", [HDK], F32, kind="ExternalInput").ap()
    bv = nc.dram_tensor("bv", [HDK], F32, kind="ExternalInput").ap()
    bo = nc.dram_tensor("bo", [D], F32, kind="ExternalInput").ap()
    gamma = nc.dram_tensor("gamma", [D], F32, kind="ExternalInput").ap()
    beta = nc.dram_tensor("beta", [D], F32, kind="ExternalInput").ap()
    y = nc.dram_tensor("y", [R, D], F32, kind="ExternalOutput").ap()

    def bcast(ap):
        # view a [D] dram vector as [128, D] (partition-broadcast read)
        return bass.AP(tensor=ap.tensor, offset=ap.offset, ap=[[0, 128]] + list(ap.ap))

    from contextlib import ExitStack
    with tile.TileContext(nc) as tc, ExitStack() as ctx:
        wts = ctx.enter_context(tc.tile_pool(name="wts", bufs=4))
        qtp = ctx.enter_context(tc.tile_pool(name="qtp", bufs=1))
        cons = ctx.enter_context(tc.tile_pool(name="cons", bufs=1))
        xrow = ctx.enter_context(tc.tile_pool(name="xrow", bufs=3))
        stat = ctx.enter_context(tc.tile_pool(name="stat", bufs=4))
        xnp = ctx.enter_context(tc.tile_pool(name="xnp", bufs=2))
        xbp = ctx.enter_context(tc.tile_pool(name="xbp", bufs=2))
        esb = ctx.enter_context(tc.tile_pool(name="esb", bufs=2))
        etp = ctx.enter_context(tc.tile_pool(name="etp", bufs=2))
        ktp = ctx.enter_context(tc.tile_pool(name="ktp", bufs=3))
        vsb = ctx.enter_context(tc.tile_pool(name="vsb", bufs=3))
        atp = ctx.enter_context(tc.tile_pool(name="atp", bufs=4))
        attp = ctx.enter_context(tc.tile_pool(name="attp", bufs=4))
        aotp = ctx.enter_context(tc.tile_pool(name="aotp", bufs=2))
        ysb = ctx.enter_context(tc.tile_pool(name="ysb", bufs=2))
        rrp = ctx.enter_context(tc.tile_pool(name="rrp", bufs=4))
        ps_pp = ctx.enter_context(tc.tile_pool(name="ps_pp", bufs=2, space="PSUM"))
        ps_sc = ctx.enter_context(tc.tile_pool(name="ps_sc", bufs=2, space="PSUM"))
        ps_ov = ctx.enter_context(tc.tile_pool(name="ps_ov", bufs=2, space="PSUM"))
        # [128, 4, 128] bf16 transpose-staging tiles (1 bank each); att
        # transposes use the first 2 slots of the same shape
        ps_tr = ctx.enter_context(tc.tile_pool(name="ps_tr", bufs=2, space="PSUM"))

        # ---- e prefetch first: pair 0/1 bf16 tiles feed the first PE work ----
        ev_v = ev.rearrange("(pr cc nj p) d -> pr p cc nj d", pr=PAIRS, cc=2, p=128)
        e2_tiles = {}
        for pr in range(2):
            e2t = esb.tile([128, 2, N, D], BF16, tag="e")
            e2_tiles[pr] = e2t
            nc.gpsimd.dma_start(out=e2t, in_=ev_v[pr])

        # ---- weights: direct f32 -> fp8e4 cast in the load DMA ----
        Wk_sb = wts.tile([128, KC, HDK], FP8, tag="w")
        nc.gpsimd.dma_start(out=Wk_sb, in_=Wk.rearrange("(kc p) n -> p kc n", p=128))
        Wv_sb = wts.tile([128, KC, HDK], FP8, tag="w")
        nc.gpsimd.dma_start(out=Wv_sb, in_=Wv.rearrange("(kc p) n -> p kc n", p=128))

        # ---- constants ----
        from concourse.masks import make_identity
        identB = cons.tile([128, 128], BF16)
        make_identity(nc, identB)
        bqc = cons.tile([128, MC], F32)
        nc.sync.dma_start(out=bqc, in_=bq.rearrange("(f p) -> p f", p=128))
        bkc = cons.tile([128, MC], F32)
        nc.sync.dma_start(out=bkc, in_=bk.rearrange("(f p) -> p f", p=128))
        bvc = cons.tile([128, MC], F32)
        nc.sync.dma_start(out=bvc, in_=bv.rearrange("(f p) -> p f", p=128))
        boB = cons.tile([128, D], F32)
        nc.gpsimd.dma_start(out=boB, in_=bcast(bo))
        gammaB = cons.tile([128, D], F32)
        nc.gpsimd.dma_start(out=gammaB, in_=bcast(gamma))
        betaB = cons.tile([128, D], F32)
        nc.gpsimd.dma_start(out=betaB, in_=bcast(beta))
        epsT = cons.tile([128, 1], F32)
        nc.vector.memset(epsT, EPS)

        Wq_sb = wts.tile([128, KC, HDK], FP8, tag="w")
        nc.gpsimd.dma_start(out=Wq_sb, in_=Wq.rearrange("(kc p) n -> p kc n", p=128))

        kv_tiles = {}

        def emit_proj(pr):
            if pr in e2_tiles:
                e2 = e2_tiles.pop(pr)
            else:
                e2 = esb.tile([128, 2, N, D], BF16, tag="e")
                nc.gpsimd.dma_start(out=e2, in_=ev_v[pr])
            # PE transpose (bf16) -> PSUM; cast to fp8 in the PSUM->SBUF copy.
            # 4 transposes per batched copy to cut copy-instruction count.
            eT = etp.tile([128, KC, 2 * TOK], FP8, tag="eT")
            for kc2 in range(KC // 2):
                for cc in range(2):
                    pt = ps_tr.tile([128, 4, 128], BF16, tag="pt")
                    for j in range(2):
                        kc = kc2 * 2 + j
                        for nj in range(N):
                            nc.tensor.transpose(
                                pt[:, j * 2 + nj, :],
                                e2[:, cc, nj, kc * 128:(kc + 1) * 128], identB)
                    nc.any.tensor_copy(
                        out=eT[:, kc2 * 2:kc2 * 2 + 2,
                               cc * TOK:(cc + 1) * TOK].rearrange(
                                   "p j (nj f) -> p j nj f", nj=2),
                        in_=pt.rearrange("p (j nj) f -> p j nj f", nj=2))

            kT = ktp.tile([128, MC, 2, TOK], BF16, tag="kT")
            for m in range(MC):
                pk = ps_pp.tile([128, 512], F32, tag="pp")
                for kk in range(KC // 2):
                    nc.tensor.matmul(pk, Wk_sb[:, 2 * kk:2 * kk + 2,
                                               m * 128:(m + 1) * 128],
                                     eT[:, 2 * kk:2 * kk + 2, :],
                                     start=(kk == 0), stop=(kk == KC // 2 - 1),
                                     perf_mode=DR)
                # bias-add on scalar engine: out = 1.0*pk + bk (per partition)
                nc.scalar.activation(out=kT[:, m, :, :].rearrange("p cc t -> p (cc t)"),
                                     in_=pk, func=Ident, bias=bkc[:, m:m + 1],
                                     scale=1.0)

            # v2 WITHOUT bias (bv folded into the attention-output copy)
            v2 = vsb.tile([128, 2, N, H, DK], BF16, tag="v")
            for cc in range(2):
                for nj in range(N):
                    for n in range(2):
                        pv = ps_pp.tile([128, 512], F32, tag="pp")
                        for kk in range(KC // 2):
                            nc.tensor.matmul(
                                pv,
                                eT[:, 2 * kk:2 * kk + 2,
                                   cc * TOK + nj * 128:cc * TOK + (nj + 1) * 128],
                                Wv_sb[:, 2 * kk:2 * kk + 2, n * 512:(n + 1) * 512],
                                start=(kk == 0), stop=(kk == KC // 2 - 1),
                                perf_mode=DR)
                        nc.any.tensor_copy(
                            out=v2[:, cc, nj, n * 8:(n + 1) * 8, :],
                            in_=pv.rearrange("p (h d) -> p h d", h=8))
            kv_tiles[pr] = (kT, v2)

        # ---- phase A: LN + transpose + q projection (emitted after pair-1
        # projections; x DMA + LN vector work overlaps pair-0/1 PE work) ----
        def emit_phase_a():
            xnT = wts.tile([128, KC, R], FP8, tag="w")
            for rt in range(R // 128):
                xa = xrow.tile([128, D], F32, tag="xrow")
                nc.sync.dma_start(out=xa, in_=x[rt * 128:(rt + 1) * 128, :])
                stats = stat.tile([128, 2, 6], F32, tag="st")
                for sg in range(2):
                    nc.vector.bn_stats(out=stats[:, sg, :],
                                       in_=xa[:, sg * 512:(sg + 1) * 512])
                mv = stat.tile([128, 2], F32, tag="mv")
                nc.vector.bn_aggr(out=mv, in_=stats)
                rstd = stat.tile([128, 1], F32, tag="rs")
                nc.scalar.activation(out=rstd, in_=mv[:, 1:2], func=Sqrt,
                                     bias=epsT, scale=1.0)
                nc.vector.reciprocal(out=rstd, in_=rstd)
                xn = xnp.tile([128, D], F32, tag="xn")
                nc.vector.tensor_scalar(out=xn, in0=xa, scalar1=mv[:, 0:1],
                                        scalar2=rstd, op0=SUB, op1=MULT)
                nc.vector.tensor_mul(out=xn, in0=xn, in1=gammaB)
                xnb = xbp.tile([128, D], BF16, tag="xnb")
                nc.vector.tensor_add(out=xnb, in0=xn, in1=betaB)
                for kc4 in range(KC // 4):
                    pt = ps_tr.tile([128, 4, 128], BF16, tag="pt")
                    for j in range(4):
                        kc = kc4 * 4 + j
                        nc.tensor.transpose(pt[:, j, :],
                                            xnb[:, kc * 128:(kc + 1) * 128], identB)
                    nc.any.tensor_copy(
                        out=xnT[:, kc4 * 4:kc4 * 4 + 4, rt * 128:(rt + 1) * 128],
                        in_=pt)

            qT = qtp.tile([128, MC, R], BF16)
            for m in range(MC):
                for n in range(2):
                    pq = ps_pp.tile([128, 512], F32, tag="pp")
                    for kk in range(KC // 2):
                        nc.tensor.matmul(pq, Wq_sb[:, 2 * kk:2 * kk + 2,
                                                   m * 128:(m + 1) * 128],
                                         xnT[:, 2 * kk:2 * kk + 2,
                                             n * 512:(n + 1) * 512],
                                         start=(kk == 0), stop=(kk == KC // 2 - 1),
                                         perf_mode=DR)
                    nc.vector.tensor_scalar(out=qT[:, m, n * 512:(n + 1) * 512],
                                            in0=pq, scalar1=bqc[:, m:m + 1],
                                            scalar2=None, op0=ADD)
            return qT

        qT_holder = {}

        def emit_attn(pr):
            qT = qT_holder["qT"]
            kT, v2 = kv_tiles.pop(pr)
            aoT = aotp.tile([128, MC, 128], FP8, tag="aoT")
            for hp in range(HP):
                ov2 = ps_ov.tile([128, 2, 128], F32, tag="ov")
                for cc in range(2):
                    cl = pr * 2 + cc
                    sc = ps_sc.tile([128, TOK], F32, tag="sc")
                    nc.tensor.matmul(sc[0:64, :], qT[0:64, hp, cl * 64:(cl + 1) * 64],
                                     kT[0:64, hp, cc, :], start=True, stop=True)
                    nc.tensor.matmul(sc[64:128, :],
                                     qT[64:128, hp, cl * 64:(cl + 1) * 64],
                                     kT[64:128, hp, cc, :], start=True, stop=True)
                    at = atp.tile([128, TOK], BF16, tag="at")
                    rs = rrp.tile([128, 1], F32, tag="rs")
                    nc.scalar.activation(out=at, in_=sc, func=Exp, scale=SCALE,
                                         accum_out=rs)
                    rr = rrp.tile([128, 1], F32, tag="rr")
                    nc.vector.reciprocal(out=rr, in_=rs)
                    nc.vector.tensor_scalar(out=at, in0=at, scalar1=rr,
                                            scalar2=None, op0=MULT)
                    att = attp.tile([128, N, 128], BF16, tag="att")
                    pt = ps_tr.tile([128, 4, 128], BF16, tag="pt")
                    for nj in range(N):
                        nc.tensor.transpose(pt[:, nj, :],
                                            at[:, nj * 128:(nj + 1) * 128], identB)
                    nc.any.tensor_copy(out=att, in_=pt[:, 0:2, :])
                    # both heads in one [128,128] matmul; off-diagonal blocks
                    # are cross-head garbage, only diagonal blocks copied out
                    for nj in range(N):
                        nc.tensor.matmul(
                            ov2[:, cc, :],
                            v2[:, cc, nj, hp * 2:hp * 2 + 2, :].rearrange(
                                "p h d -> p (h d)"),
                            att[:, nj, :],
                            start=(nj == 0), stop=(nj == N - 1))
                # diagonal-block copy for both chunks at once, + bv bias
                # (valid because att rows are normalized), fp8 output
                for h01 in range(2):
                    sl = slice(h01 * 64, (h01 + 1) * 64)
                    nc.vector.tensor_scalar(
                        out=aoT[sl, hp, :].rearrange("p (cc i) -> p cc i", cc=2),
                        in0=ov2[sl, :, sl],
                        scalar1=bvc[sl, hp:hp + 1], scalar2=None, op0=ADD)

            xres = xrow.tile([128, D], F32, tag="xrow")
            nc.sync.dma_start(out=xres, in_=x[pr * 128:(pr + 1) * 128, :])
            y_sb = ysb.tile([128, D], F32, tag="y")
            for n in range(2):
                py = ps_pp.tile([128, 512], F32, tag="pp")
                for kk in range(MC // 2):
                    nc.tensor.matmul(py, aoT[:, 2 * kk:2 * kk + 2, :],
                                     Wo_sb[:, 2 * kk:2 * kk + 2,
                                           n * 512:(n + 1) * 512],
                                     start=(kk == 0), stop=(kk == MC // 2 - 1),
                                     perf_mode=DR)
                nc.vector.tensor_add(out=y_sb[:, n * 512:(n + 1) * 512], in0=py,
                                     in1=boB[:, n * 512:(n + 1) * 512])
            nc.vector.tensor_add(out=y_sb, in0=y_sb, in1=xres)
            nc.sync.dma_start(out=y[pr * 128:(pr + 1) * 128, :], in_=y_sb)

        # ---- schedule ----
        emit_proj(0)
        emit_proj(1)
        qT_holder["qT"] = emit_phase_a()
        # Wo reuses a weight slot (free after q-proj's xnT... keep separate tag)
        Wo_sb = wts.tile([128, MC, D], FP8, tag="wo")
        nc.gpsimd.dma_start(out=Wo_sb, in_=Wo.rearrange("(mc p) n -> p mc n", p=128))
        for pr in range(2, PAIRS):
            emit_proj(pr)
            emit_attn(pr - 2)
        emit_attn(PAIRS - 2)
        emit_attn(PAIRS - 1)

    nc.compile()
    return nc


_NC = None


def _get_nc():
    global _NC
    if _NC is None:
        _NC = build_bass()
    return _NC


def _shard_inputs(h, e, Wq, bq, Wk, bk, Wv, bv, Wo, bo, gamma, beta):
    shared = {"Wq": Wq, "Wk": Wk, "Wv": Wv, "Wo": Wo, "bq": bq, "bk": bk,
              "bv": bv, "bo": bo, "gamma": gamma, "beta": beta}
    in_maps = []
    for r in range(8):
        b, half = divmod(r, 2)
        c0 = half * CPC
        t0 = CHUNK - 1 + c0 * CHUNK
        rows = h[b, t0:min(t0 + R, S)]
        if rows.shape[0] < R:
            rows = np.concatenate(
                [rows, np.zeros((R - rows.shape[0], D), np.float32)], axis=0)
        evs = np.ascontiguousarray(e[b, c0:c0 + CPC].reshape(CPC * TOK, D))
        in_maps.append({"x": np.ascontiguousarray(rows), "ev": evs, **shared})
    return in_maps


# results of the most recent run (exec_time_ns etc.) for test harnesses
LAST_RESULTS = None
TRACE = False


def kernel(h, e, Wq, bq, Wk, bk, Wv, bv, Wo, bo, gamma, beta):
    global LAST_RESULTS
    args = [np.asarray(a, dtype=np.float32) for a in
            (h, e, Wq, bq, Wk, bk, Wv, bv, Wo, bo, gamma, beta)]
    h, e = args[0], args[1]
    nc = _get_nc()
    in_maps = _shard_inputs(*args)
    res = run_bass_kernel_spmd(nc, in_maps, core_ids=list(range(8)), trace=TRACE)
    LAST_RESULTS = res
    out = np.empty((B, S, D), np.float32)
    out[:, :CHUNK - 1] = h[:, :CHUNK - 1]
    for r in range(8):
        b, half = divmod(r, 2)
        c0 = half * CPC
        t0 = CHUNK - 1 + c0 * CHUNK
        n = min(R, S - t0)
        out[b, t0:t0 + n] = res.results[r]["y"][:n]
    return out
